# revision 43
# baseline (speedup 1.0000x reference)
"""GRU decoder (nn_Decoder) Trainium2 Bass kernel.

Strategy: pure data parallelism — batch B=8192 sharded over 8 NeuronCores
(1024 rows each), weights replicated. Features live on the partition axis
(h.T is [H, B_c]), so the GRU recurrence is stationary-weight PE matmuls
streaming the batch.

Per core / per step (batch split in 2 chunks of 512):
  - All gate pre-activations come from fp8e4 DoubleRow matmuls (cost-model
    0.5 cycles/row): the K=256 recurrent contraction fits one DR matmul per
    128-row gate tile, and the input side is a one-hot DR matmul (K=32 =
    16x2) against a per-token table with the gate biases pre-folded into
    every row (b_hh_n rides extra table columns accumulated onto phn).
    The z-gate weights are negated host-side so its sigmoid yields
    zc = 1 - z, enabling cheaper combine forms.
  - r/z: one sigmoid ACT op per gate per chunk over the accumulated PSUM.
  - n: npre = (phn + b_hh_n) * r on the DVE (scalar_tensor_tensor; GPSIMD
    cannot access PSUM), an identity matmul accumulates npre onto the i_n
    PSUM bank, tanh reads PSUM.
  - combine on the DVE (bf16 tensor_tensor, 2x mode):
      chunk 0: h' = zc*n + (h - zc*h)  -- two ops run before tanh
      chunk 1: h' = h - zc*(h - n)     -- all post-tanh
    plus a bf16->fp8 copy of h for the next step's DR matmuls.
  - Output projections run bf16 on the PE (p1 -> [32, 8, 128]; no
    tile_position anywhere: row-packed matmuls crash the runtime), relu
    split between ACT and DVE, results staged in SBUF and DMA'd every 13
    steps in a host-friendly layout (transpose + bp2 add on the host).
"""

import numpy as np
import ml_dtypes

B, L, H, A, T, E = 8192, 128, 256, 32, 65, 8
NCORES = 8
BC = B // NCORES          # 1024 batch rows per core
NCH = 2                   # batch chunks per step
CH = BC // NCH            # 512
G3 = 3 * H                # 768
NST = 13                  # steps per output stage / oh prefetch group
NGRP = T // NST           # 5

BF16 = ml_dtypes.bfloat16
FP8 = ml_dtypes.float8_e4m3fn

_CACHE = {}

# tuning knobs (see sweep)
PROJ_TAG = "pr"
P1_LEAD = True
ACT_ORDER = "A"
D1_GPS = False
CONV0_GPS = False
CONV1_GPS = True
RELU_ENG = "split"
HP_BUFS = 2
WORK_BUFS = 2
FORM0 = 5
FORM1 = 3
QS0_GPS = False
DIRECT_DMA = False
M1SPLIT = False
STAGE_ACT = False
HN8C0 = False
HN8C1 = False


def _build(trace=False):
    import concourse.bass as bass
    import concourse.bacc as bacc
    import concourse.tile as tile
    from concourse import mybir
    from contextlib import ExitStack

    f32 = mybir.dt.float32
    bf16 = mybir.dt.bfloat16
    fp8 = mybir.dt.float8e4
    Alu = mybir.AluOpType
    Act = mybir.ActivationFunctionType
    PM = mybir.MatmulPerfMode

    nc = bacc.Bacc("TRN2", target_bir_lowering=False, debug=False)

    lat = nc.dram_tensor("lat", [128, BC], bf16, kind="ExternalInput")
    oh = nc.dram_tensor("oh", [16, 2, T, BC], fp8, kind="ExternalInput")
    whh = nc.dram_tensor("whh", [128, 2, G3], fp8, kind="ExternalInput")
    giv = nc.dram_tensor("giv", [16, 2, G3 + H], fp8, kind="ExternalInput")
    wd0 = nc.dram_tensor("wd0", [128, H], bf16, kind="ExternalInput")
    wd1 = nc.dram_tensor("wd1", [128, 2, H], bf16, kind="ExternalInput")
    wd2 = nc.dram_tensor("wd2", [128, 2, H], bf16, kind="ExternalInput")
    wp1 = nc.dram_tensor("wp1", [128, 2, A], bf16, kind="ExternalInput")
    wp2 = nc.dram_tensor("wp2", [128, A], bf16, kind="ExternalInput")
    bias = nc.dram_tensor("bias", [128, 10], f32, kind="ExternalInput")
    iden = nc.dram_tensor("iden", [128, 128], bf16, kind="ExternalInput")
    # outw: [128 partitions, T, NCH*4*A]; host reassembles [BC, T, A]
    outw = nc.dram_tensor("outw", [128, T, NCH * 4 * A], f32,
                          kind="ExternalOutput")

    with ExitStack() as ctx:
        tc = ctx.enter_context(tile.TileContext(nc))
        const = ctx.enter_context(tc.tile_pool(name="const", bufs=1))
        hp = ctx.enter_context(tc.tile_pool(name="hp", bufs=HP_BUFS))
        work = ctx.enter_context(tc.tile_pool(name="work", bufs=WORK_BUFS))
        ohp = ctx.enter_context(tc.tile_pool(name="ohp", bufs=2))
        outp = ctx.enter_context(tc.tile_pool(name="outp", bufs=2))
        psum = ctx.enter_context(tc.tile_pool(name="psum", bufs=1, space="PSUM"))

        # ---- load constants ----
        lat_sb = const.tile([128, BC], bf16, tag="lat")
        nc.sync.dma_start(out=lat_sb[:], in_=lat[:])
        whh_sb = const.tile([128, 2, G3], fp8, tag="whh")
        nc.sync.dma_start(out=whh_sb[:], in_=whh[:])
        giv_sb = const.tile([16, 2, G3 + H], fp8, tag="giv")
        nc.sync.dma_start(out=giv_sb[:], in_=giv[:])
        wd0_sb = const.tile([128, H], bf16, tag="wd0")
        nc.sync.dma_start(out=wd0_sb[:], in_=wd0[:])
        wd1_sb = const.tile([128, 2, H], bf16, tag="wd1")
        nc.sync.dma_start(out=wd1_sb[:], in_=wd1[:])
        wd2_sb = const.tile([128, 2, H], bf16, tag="wd2")
        nc.sync.dma_start(out=wd2_sb[:], in_=wd2[:])
        wp1_sb = const.tile([128, 2, A], bf16, tag="wp1")
        nc.sync.dma_start(out=wp1_sb[:], in_=wp1[:])
        wp2_sb = const.tile([128, A], bf16, tag="wp2")
        nc.sync.dma_start(out=wp2_sb[:], in_=wp2[:])
        bias_sb = const.tile([128, 10], f32, tag="bias")
        nc.sync.dma_start(out=bias_sb[:], in_=bias[:])
        iden_sb = const.tile([128, 128], bf16, tag="iden")
        nc.sync.dma_start(out=iden_sb[:], in_=iden[:])

        # bias_sb columns: 0,1: b_hh_n (m0, m1); 2: bp1 (x4); 3: bp2 (x4);
        # 4..9: bd0 m0, bd0 m1, bd1 m0, bd1 m1, bd2 m0, bd2 m1

        # ---- MLP prologue (bf16): h0 = mlp(latent) ----
        h_cur = hp.tile([128, 2, BC], bf16, tag="h", name="h_init")
        h8_cur = hp.tile([128, 2, BC], fp8, tag="h8", name="h8_init")
        h1 = work.tile([128, 2, BC], bf16, tag="mlpa", name="mlpa")
        h2 = work.tile([128, 2, BC], bf16, tag="mlpb", name="mlpb")
        for c in range(NCH):
            cs = slice(c * CH, (c + 1) * CH)
            ps = psum.tile([128, 2, CH], f32, tag="pr", bufs=2, name=f"mlp1_{c}")
            for m in range(2):
                nc.tensor.matmul(
                    ps[:, m, :], wd0_sb[:, m * 128:(m + 1) * 128], lat_sb[:, cs],
                    start=True, stop=True)
            for m in range(2):
                nc.scalar.activation(
                    out=h1[:, m, cs], in_=ps[:, m, :], func=Act.Relu,
                    bias=bias_sb[:, 4 + m:5 + m])
        for c in range(NCH):
            cs = slice(c * CH, (c + 1) * CH)
            ps = psum.tile([128, 2, CH], f32, tag="pr", bufs=2, name=f"mlp2_{c}")
            for m in range(2):
                for kc in range(2):
                    nc.tensor.matmul(
                        ps[:, m, :], wd1_sb[:, kc, m * 128:(m + 1) * 128],
                        h1[:, kc, cs], start=(kc == 0), stop=(kc == 1))
            for m in range(2):
                nc.scalar.activation(
                    out=h2[:, m, cs], in_=ps[:, m, :], func=Act.Relu,
                    bias=bias_sb[:, 6 + m:7 + m])
        for c in range(NCH):
            cs = slice(c * CH, (c + 1) * CH)
            ps = psum.tile([128, 2, CH], f32, tag="pr", bufs=2, name=f"mlp3_{c}")
            for m in range(2):
                for kc in range(2):
                    nc.tensor.matmul(
                        ps[:, m, :], wd2_sb[:, kc, m * 128:(m + 1) * 128],
                        h2[:, kc, cs], start=(kc == 0), stop=(kc == 1))
            for m in range(2):
                nc.scalar.activation(
                    out=h_cur[:, m, cs], in_=ps[:, m, :], func=Act.Identity,
                    bias=bias_sb[:, 8 + m:9 + m])
            nc.vector.tensor_copy(h8_cur[:, :, cs], h_cur[:, :, cs])

        # ---- GRU steps ----
        def emit_proj_p1(h_tile, tp):
            """p1 matmuls for step tp (pipelined 1 behind): out [32, 8, 128]."""
            p1ps = psum.tile([32, 8, 128], f32, tag=PROJ_TAG, bufs=2,
                             name=f"p1ps_{tp}")
            for j in range(8):
                bs = slice(j * 128, (j + 1) * 128)
                for kc in range(2):
                    nc.tensor.matmul(
                        p1ps[:, j, :], wp1_sb[:, kc, :], h_tile[:, kc, bs],
                        start=(kc == 0), stop=(kc == 1))
            return p1ps

        def emit_proj_relu(p1ps, tp):
            p1t = work.tile([32, 8, 128], bf16, tag="p1t", name=f"p1t_{tp}")
            if RELU_ENG == "act":
                nc.scalar.activation(out=p1t[:], in_=p1ps[:], func=Act.Relu,
                                     bias=bias_sb[0:32, 2:3])
            elif RELU_ENG == "split":
                nc.scalar.activation(out=p1t[:, 0:4, :], in_=p1ps[:, 0:4, :],
                                     func=Act.Relu, bias=bias_sb[0:32, 2:3])
                nc.vector.tensor_scalar(
                    out=p1t[:, 4:8, :], in0=p1ps[:, 4:8, :],
                    scalar1=bias_sb[0:32, 2:3], scalar2=0.0,
                    op0=Alu.add, op1=Alu.max)
            else:
                nc.vector.tensor_scalar(
                    out=p1t[:], in0=p1ps[:], scalar1=bias_sb[0:32, 2:3],
                    scalar2=0.0, op0=Alu.add, op1=Alu.max)
            return p1t

        def emit_proj_p2(p1t, tp, stage):
            """p2 matmuls + DVE stage copy for step tp."""
            p2ps = psum.tile([128, 8, A], f32, tag=PROJ_TAG, bufs=2,
                             name=f"p2ps_{tp}")
            for j in range(8):
                nc.tensor.matmul(
                    p2ps[:, j, :], p1t[:, j, :], wp2_sb[0:A, :],
                    start=True, stop=True)
            if STAGE_ACT:
                nc.scalar.activation(
                    out=stage[:, tp % NST, :],
                    in_=p2ps.rearrange("p j a -> p (j a)"), func=Act.Identity)
            else:
                nc.vector.tensor_copy(
                    stage[:, tp % NST, :], p2ps.rearrange("p j a -> p (j a)"))

        def gate_mms(pr, pz, phn, pin, oh_rhs, h8_rhs):
            for m in range(2):
                nc.tensor.matmul(
                    pr[:, m, :], giv_sb[:, :, m * 128:(m + 1) * 128],
                    oh_rhs, start=True, stop=False, perf_mode=PM.DoubleRow)
                nc.tensor.matmul(
                    pr[:, m, :], whh_sb[:, :, m * 128:(m + 1) * 128],
                    h8_rhs, start=False, stop=True, perf_mode=PM.DoubleRow)
            for m in range(2):
                nc.tensor.matmul(
                    pz[:, m, :], giv_sb[:, :, 256 + m * 128:256 + (m + 1) * 128],
                    oh_rhs, start=True, stop=False, perf_mode=PM.DoubleRow)
                nc.tensor.matmul(
                    pz[:, m, :], whh_sb[:, :, 256 + m * 128:256 + (m + 1) * 128],
                    h8_rhs, start=False, stop=True, perf_mode=PM.DoubleRow)
            for m in range(2):
                nc.tensor.matmul(
                    phn[:, m, :], whh_sb[:, :, 512 + m * 128:512 + (m + 1) * 128],
                    h8_rhs, start=True, stop=False, perf_mode=PM.DoubleRow)
                nc.tensor.matmul(
                    phn[:, m, :], giv_sb[:, :, 768 + m * 128:768 + (m + 1) * 128],
                    oh_rhs, start=False, stop=True, perf_mode=PM.DoubleRow)
            for m in range(2):
                nc.tensor.matmul(
                    pin[:, m, :], giv_sb[:, :, 512 + m * 128:512 + (m + 1) * 128],
                    oh_rhs, start=True, stop=False, perf_mode=PM.DoubleRow)

        h_prev = None
        p1t_prev = None
        oh_g = None
        stages = {}
        for t in range(T):
            g, ti = divmod(t, NST)
            if ti == 0:
                oh_g = ohp.tile([16, 2, NST, BC], fp8, tag="ohg",
                                name=f"ohg_{g}")
                nc.sync.dma_start(out=oh_g[:], in_=oh[:, :, g * NST:(g + 1) * NST, :])
                stages[g] = outp.tile([128, NST, NCH * 4 * A], f32, tag="stage",
                                      name=f"stage_{g}")

            h_new = hp.tile([128, 2, BC], bf16, tag="h", name=f"h_{t}")
            h8_new = hp.tile([128, 2, BC], fp8, tag="h8", name=f"h8_{t}")
            rz = work.tile([128, 4, BC], bf16, tag="rz", name=f"rz_{t}")
            npre = work.tile([128, 2, BC], bf16, tag="npre", name=f"npre_{t}")
            nsb = work.tile([128, 2, BC], bf16, tag="nsb", name=f"nsb_{t}")
            t3 = work.tile([128, 2, BC], bf16, tag="t3", name=f"t3_{t}")
            t4 = work.tile([128, 2, BC], bf16, tag="t4", name=f"t4_{t}")
            c0s, c1s = slice(0, CH), slice(CH, BC)

            if P1_LEAD and h_prev is not None:
                p1ps_prev = emit_proj_p1(h_prev, t - 1)

            # NOTE: z weights are negated host-side -> sigmoid yields
            # zc = 1-z:  h_new = zc*n + (h - zc*h)   (c0, early q/s)
            #            h_new = h - zc*(h - n)      (c1, post-tanh)
            pr0 = psum.tile([128, 2, CH], f32, tag="pr", bufs=2, name=f"pr_{t}_0")
            pz0 = psum.tile([128, 2, CH], f32, tag="pr", bufs=2, name=f"pz_{t}_0")
            phn0 = psum.tile([128, 2, CH], f32, tag="pnp", bufs=2,
                             name=f"phn_{t}_0")
            pin0 = psum.tile([128, 2, CH], f32, tag="pnp", bufs=2,
                             name=f"pin_{t}_0")
            gate_mms(pr0, pz0, phn0, pin0, oh_g[:, :, ti, c0s],
                     h8_cur[:, :, c0s])
            pr1 = psum.tile([128, 2, CH], f32, tag="pr", bufs=2, name=f"pr_{t}_1")
            pz1 = psum.tile([128, 2, CH], f32, tag="pr", bufs=2, name=f"pz_{t}_1")
            phn1 = psum.tile([128, 2, CH], f32, tag="pnp", bufs=2,
                             name=f"phn_{t}_1")
            pin1 = psum.tile([128, 2, CH], f32, tag="pnp", bufs=2,
                             name=f"pin_{t}_1")
            gate_mms(pr1, pz1, phn1, pin1, oh_g[:, :, ti, c1s],
                     h8_cur[:, :, c1s])
            # proj(t-1) p1 matmuls go behind the gate matmuls on the PE
            if not P1_LEAD and h_prev is not None:
                p1ps_prev = emit_proj_p1(h_prev, t - 1)

            def npre_c(ci, phn, cs):
                nc.vector.scalar_tensor_tensor(
                    out=npre[:, :, cs], in0=phn[:], scalar=0.0,
                    in1=rz[:, 0:2, cs], op0=Alu.add, op1=Alu.mult)

            def ident_c(pin, cs):
                for m in range(2):
                    nc.tensor.matmul(pin[:, m, :], iden_sb[:], npre[:, m, cs],
                                     start=False, stop=True)

            def sig_r0():
                nc.scalar.activation(out=rz[:, 0:2, c0s], in_=pr0[:], func=Act.Sigmoid)
            def sig_z0():
                nc.scalar.activation(out=rz[:, 2:4, c0s], in_=pz0[:], func=Act.Sigmoid)
            def sig_r1():
                nc.scalar.activation(out=rz[:, 0:2, c1s], in_=pr1[:], func=Act.Sigmoid)
            def sig_z1():
                nc.scalar.activation(out=rz[:, 2:4, c1s], in_=pz1[:], func=Act.Sigmoid)
            def tanh0():
                npre_c(0, phn0, c0s)
                ident_c(pin0, c0s)
                nc.scalar.activation(out=nsb[:, :, c0s], in_=pin0[:], func=Act.Tanh)
            def tanh1():
                npre_c(1, phn1, c1s)
                ident_c(pin1, c1s)
                if M1SPLIT:
                    for m in range(2):
                        nc.scalar.activation(out=nsb[:, m, c1s],
                                             in_=pin1[:, m, :], func=Act.Tanh)
                else:
                    nc.scalar.activation(out=nsb[:, :, c1s], in_=pin1[:],
                                         func=Act.Tanh)

            ORDERS = {
                "A": [sig_r0, sig_z0, tanh0, sig_r1, sig_z1, tanh1],
                "B": [sig_r0, sig_z0, sig_r1, tanh0, sig_z1, tanh1],
                "C": [sig_r0, sig_z0, sig_r1, sig_z1, tanh0, tanh1],
                "D": [sig_r0, sig_r1, sig_z0, sig_z1, tanh0, tanh1],
                "E": [sig_r0, sig_r1, sig_z0, tanh0, sig_z1, tanh1],
            }
            for fn in ORDERS[ACT_ORDER]:
                fn()

            # chunk 0 combine
            def combine(form, cs_, conv_gps):
                if form == 5:
                    # q = zc*h, s = h - q (early);  u = zc*n, h = u + s (late)
                    if QS0_GPS:
                        nc.gpsimd.scalar_tensor_tensor(
                            out=t3[:, :, cs_], in0=h_cur[:, :, cs_], scalar=0.0,
                            in1=rz[:, 2:4, cs_], op0=Alu.add, op1=Alu.mult)
                        nc.gpsimd.scalar_tensor_tensor(
                            out=t3[:, :, cs_], in0=h_cur[:, :, cs_], scalar=0.0,
                            in1=t3[:, :, cs_], op0=Alu.add, op1=Alu.subtract)
                    else:
                        nc.vector.tensor_mul(t3[:, :, cs_], rz[:, 2:4, cs_], h_cur[:, :, cs_])
                        nc.vector.tensor_sub(t3[:, :, cs_], h_cur[:, :, cs_], t3[:, :, cs_])
                    nc.vector.tensor_mul(t4[:, :, cs_], rz[:, 2:4, cs_], nsb[:, :, cs_])
                    if HN8C0:
                        nc.vector.tensor_add(h8_new[:, :, cs_], t4[:, :, cs_],
                                             t3[:, :, cs_])
                        nc.gpsimd.scalar_tensor_tensor(
                            out=h_new[:, :, cs_], in0=t4[:, :, cs_], scalar=0.0,
                            in1=t3[:, :, cs_], op0=Alu.add, op1=Alu.add)
                        return
                    nc.vector.tensor_add(h_new[:, :, cs_], t4[:, :, cs_], t3[:, :, cs_])
                else:
                    # h_new = h - zc*(h - n)
                    nc.vector.tensor_sub(t3[:, :, cs_], h_cur[:, :, cs_], nsb[:, :, cs_])
                    nc.vector.tensor_mul(t3[:, :, cs_], rz[:, 2:4, cs_], t3[:, :, cs_])
                    if HN8C1:
                        nc.vector.tensor_sub(h8_new[:, :, cs_], h_cur[:, :, cs_],
                                             t3[:, :, cs_])
                        nc.gpsimd.scalar_tensor_tensor(
                            out=h_new[:, :, cs_], in0=h_cur[:, :, cs_], scalar=0.0,
                            in1=t3[:, :, cs_], op0=Alu.add, op1=Alu.subtract)
                        return
                    nc.vector.tensor_sub(h_new[:, :, cs_], h_cur[:, :, cs_], t3[:, :, cs_])
                if conv_gps:
                    nc.gpsimd.tensor_copy(h8_new[:, :, cs_], h_new[:, :, cs_])
                else:
                    nc.vector.tensor_copy(h8_new[:, :, cs_], h_new[:, :, cs_])

            combine(FORM0, c0s, CONV0_GPS)

            # proj(t-1) p2 + stage
            if h_prev is not None:
                p1t_prev = emit_proj_relu(p1ps_prev, t - 1)
                emit_proj_p2(p1t_prev, t - 1, stages[(t - 1) // NST])
                if not DIRECT_DMA and (t - 1) % NST == NST - 1:
                    gg = (t - 1) // NST
                    nc.sync.dma_start(
                        out=outw[:, gg * NST:(gg + 1) * NST, :],
                        in_=stages[gg][:])

            # chunk 1 combine
            if M1SPLIT:
                for m in range(2):
                    ms = (slice(None), m, c1s)
                    nc.vector.tensor_sub(t3[ms], h_cur[ms], nsb[ms])
                    nc.vector.tensor_mul(t3[ms], rz[:, 2 + m, c1s], t3[ms])
                    nc.vector.tensor_sub(h_new[ms], h_cur[ms], t3[ms])
                    if CONV1_GPS:
                        nc.gpsimd.tensor_copy(h8_new[ms], h_new[ms])
                    else:
                        nc.vector.tensor_copy(h8_new[ms], h_new[ms])
            else:
                combine(FORM1, c1s, CONV1_GPS)

            h_prev = h_new
            h_cur = h_new
            h8_cur = h8_new
        p1ps_last = emit_proj_p1(h_prev, T - 1)
        p1t_last = emit_proj_relu(p1ps_last, T - 1)
        emit_proj_p2(p1t_last, T - 1, stages[NGRP - 1])
        if not DIRECT_DMA:
            nc.sync.dma_start(
                out=outw[:, (NGRP - 1) * NST:, :], in_=stages[NGRP - 1][:])

    nc.finalize()
    return nc


def _prep_inputs(latent, target, embed, W_ih, b_ih, W_hh, b_hh,
                 Wd0, bd0, Wd1, bd1, Wd2, bd2, Wp1, bp1, Wp2, bp2):
    f = np.float32
    latent = np.asarray(latent, dtype=f)
    embed = np.asarray(embed, dtype=f)
    W_ih = np.asarray(W_ih, dtype=f)
    b_ih = np.asarray(b_ih, dtype=f)
    W_hh = np.asarray(W_hh, dtype=f)
    b_hh = np.asarray(b_hh, dtype=f)

    # tokens with teacher-forcing shift
    tokens = np.concatenate(
        [np.zeros((B, 1), dtype=np.int64),
         np.asarray(target[:, :-1], dtype=np.int64)], axis=1)  # [B, T]

    # per-token gate table with biases folded in:
    #   r/z rows: giv + b_ih + b_hh ; n rows: giv + b_ih
    # plus H extra columns: b_hh_n broadcast to every token (accumulated
    # onto phn by the one-hot matmul). The z columns are NEGATED so the
    # sigmoid yields zc = 1 - z.
    giv = embed @ W_ih.T  # [A, 3H]
    gt = np.concatenate([giv, np.zeros((A, H), dtype=f)], axis=1)
    gt[:, :2 * H] += (b_ih + b_hh)[None, :2 * H]
    gt[:, 2 * H:3 * H] += b_ih[None, 2 * H:]
    gt[:, H:2 * H] *= -1.0
    gt[:, 3 * H:] = b_hh[None, 2 * H:]
    # DR layout [16, 2, 3H+H]: table row a = (a % 16) + 16 * (a // 16)
    giv_dr = np.ascontiguousarray(
        gt.reshape(2, 16, G3 + H).transpose(1, 0, 2)).astype(FP8)

    # one-hot, DR layout [16, 2, T, B]
    tok_tm = tokens.T  # [T, B]
    ohf = np.zeros((16, 2, T, B), dtype=FP8)
    for a in range(A):
        ohf[a % 16, a // 16][tok_tm == a] = 1.0

    whhT = np.ascontiguousarray(W_hh.T).copy()  # [H, 3H]
    whhT[:, H:2 * H] *= -1.0  # negated z gate -> sigmoid gives 1-z
    whh_dr = np.ascontiguousarray(
        whhT.reshape(2, 128, G3).transpose(1, 0, 2)).astype(FP8)

    wd0_l = np.asarray(Wd0, dtype=f).astype(BF16)                   # [128, 256]
    wd1_l = np.ascontiguousarray(
        np.asarray(Wd1, dtype=f).reshape(2, 128, H).transpose(1, 0, 2)).astype(BF16)
    wd2_l = np.ascontiguousarray(
        np.asarray(Wd2, dtype=f).reshape(2, 128, H).transpose(1, 0, 2)).astype(BF16)
    wp1_l = np.ascontiguousarray(
        np.asarray(Wp1, dtype=f).reshape(2, 128, A).transpose(1, 0, 2)).astype(BF16)
    wp2_l = np.zeros((128, A), dtype=f)
    wp2_l[:A] = np.asarray(Wp2, dtype=f)
    wp2_l = wp2_l.astype(BF16)                                      # [128, 32]

    bias_pack = np.zeros((128, 10), dtype=f)
    bias_pack[:, 0] = b_hh[2 * H: 2 * H + 128]
    bias_pack[:, 1] = b_hh[2 * H + 128:]
    bias_pack[:A, 2] = np.asarray(bp1, dtype=f)
    bias_pack[:, 3] = np.tile(np.asarray(bp2, dtype=f), 4)  # per (j,a) flattened
    bias_pack[:, 4] = np.asarray(bd0, dtype=f)[:128]
    bias_pack[:, 5] = np.asarray(bd0, dtype=f)[128:]
    bias_pack[:, 6] = np.asarray(bd1, dtype=f)[:128]
    bias_pack[:, 7] = np.asarray(bd1, dtype=f)[128:]
    bias_pack[:, 8] = np.asarray(bd2, dtype=f)[:128]
    bias_pack[:, 9] = np.asarray(bd2, dtype=f)[128:]

    iden = np.eye(128, dtype=f).astype(BF16)

    latT = np.ascontiguousarray(latent.T).astype(BF16)  # [128, B]

    shared = dict(whh=whh_dr, giv=giv_dr, wd0=wd0_l, wd1=wd1_l, wd2=wd2_l,
                  wp1=wp1_l, wp2=wp2_l, bias=bias_pack, iden=iden)
    in_maps = []
    for cid in range(NCORES):
        bs = slice(cid * BC, (cid + 1) * BC)
        m = dict(shared)
        m["lat"] = np.ascontiguousarray(latT[:, bs])
        m["oh"] = np.ascontiguousarray(ohf[:, :, :, bs])
        in_maps.append(m)
    return in_maps


def _unpack_out(outw, bp2):
    """outw [128, T, 8*A] f32 -> [BC, T, A] + bp2."""
    o = outw.reshape(128, T, 8, A)               # p, t, j, a
    o = o.transpose(2, 0, 1, 3)                  # j, p, t, a
    o = np.ascontiguousarray(o.reshape(BC, T, A))
    o += np.asarray(bp2, dtype=np.float32)[None, None, :]
    return o


def kernel(**inputs):
    from concourse.bass_utils import run_bass_kernel_spmd

    if "nc" not in _CACHE:
        _CACHE["nc"] = _build()
    nc = _CACHE["nc"]

    in_maps = _prep_inputs(**inputs)
    res = run_bass_kernel_spmd(nc, in_maps, core_ids=list(range(NCORES)))
    bp2 = inputs["bp2"]
    outs = [_unpack_out(r["outw"], bp2) for r in res.results]
    return np.concatenate(outs, axis=0).astype(np.float32)


# revision 44
# speedup vs baseline: 1.0047x; 1.0047x over previous
"""GRU decoder (nn_Decoder) Trainium2 Bass kernel.

Strategy: pure data parallelism — batch B=8192 sharded over 8 NeuronCores
(1024 rows each), weights replicated. Features live on the partition axis
(h.T is [H, B_c]), so the GRU recurrence is stationary-weight PE matmuls
streaming the batch.

Per core / per step (batch split in 2 chunks of 512):
  - All gate pre-activations come from fp8e4 DoubleRow matmuls (cost-model
    0.5 cycles/row): the K=256 recurrent contraction fits one DR matmul per
    128-row gate tile, and the input side is a one-hot DR matmul (K=32 =
    16x2) against a per-token table with the gate biases pre-folded into
    every row (b_hh_n rides extra table columns accumulated onto phn).
    The z-gate weights are negated host-side so its sigmoid yields
    zc = 1 - z, enabling cheaper combine forms.
  - r/z: one sigmoid ACT op per gate per chunk over the accumulated PSUM.
  - n: npre = (phn + b_hh_n) * r on the DVE (scalar_tensor_tensor; GPSIMD
    cannot access PSUM), an identity matmul accumulates npre onto the i_n
    PSUM bank, tanh reads PSUM.
  - combine on the DVE (bf16 tensor_tensor, 2x mode):
      chunk 0: h' = zc*n + (h - zc*h)  -- two ops run before tanh
      chunk 1: h' = h - zc*(h - n)     -- all post-tanh
    plus a bf16->fp8 copy of h for the next step's DR matmuls.
  - Output projections run bf16 on the PE (p1 -> [32, 8, 128]; no
    tile_position anywhere: row-packed matmuls crash the runtime), relu
    split between ACT and DVE, results staged in SBUF and DMA'd every 13
    steps in a host-friendly layout (transpose + bp2 add on the host).
"""

import numpy as np
import ml_dtypes

B, L, H, A, T, E = 8192, 128, 256, 32, 65, 8
NCORES = 8
BC = B // NCORES          # 1024 batch rows per core
NCH = 2                   # batch chunks per step
CH = BC // NCH            # 512
G3 = 3 * H                # 768
NST = 5                   # steps per output stage / oh prefetch group
NGRP = T // NST           # 13

BF16 = ml_dtypes.bfloat16
FP8 = ml_dtypes.float8_e4m3fn

_CACHE = {}

# tuning knobs (see sweep)
PROJ_TAG = "pr"
P1_LEAD = True
ACT_ORDER = "A"
D1_GPS = False
CONV0_GPS = False
CONV1_GPS = True
RELU_ENG = "split"
HP_BUFS = 2
WORK_BUFS = 2
FORM0 = 5
FORM1 = 3
QS0_GPS = False
DIRECT_DMA = False
M1SPLIT = False
STAGE_ACT = False
HN8C0 = False
HN8C1 = False


def _build(trace=False):
    import concourse.bass as bass
    import concourse.bacc as bacc
    import concourse.tile as tile
    from concourse import mybir
    from contextlib import ExitStack

    f32 = mybir.dt.float32
    bf16 = mybir.dt.bfloat16
    fp8 = mybir.dt.float8e4
    Alu = mybir.AluOpType
    Act = mybir.ActivationFunctionType
    PM = mybir.MatmulPerfMode

    nc = bacc.Bacc("TRN2", target_bir_lowering=False, debug=False)

    lat = nc.dram_tensor("lat", [128, BC], bf16, kind="ExternalInput")
    oh = nc.dram_tensor("oh", [16, 2, T, BC], fp8, kind="ExternalInput")
    whh = nc.dram_tensor("whh", [128, 2, G3], fp8, kind="ExternalInput")
    giv = nc.dram_tensor("giv", [16, 2, G3 + H], fp8, kind="ExternalInput")
    wd0 = nc.dram_tensor("wd0", [128, H], bf16, kind="ExternalInput")
    wd1 = nc.dram_tensor("wd1", [128, 2, H], bf16, kind="ExternalInput")
    wd2 = nc.dram_tensor("wd2", [128, 2, H], bf16, kind="ExternalInput")
    wp1 = nc.dram_tensor("wp1", [128, 2, A], bf16, kind="ExternalInput")
    wp2 = nc.dram_tensor("wp2", [128, A], bf16, kind="ExternalInput")
    bias = nc.dram_tensor("bias", [128, 10], f32, kind="ExternalInput")
    iden = nc.dram_tensor("iden", [128, 128], bf16, kind="ExternalInput")
    # outw: [128 partitions, T, NCH*4*A]; host reassembles [BC, T, A]
    outw = nc.dram_tensor("outw", [128, T, NCH * 4 * A], f32,
                          kind="ExternalOutput")

    with ExitStack() as ctx:
        tc = ctx.enter_context(tile.TileContext(nc))
        const = ctx.enter_context(tc.tile_pool(name="const", bufs=1))
        hp = ctx.enter_context(tc.tile_pool(name="hp", bufs=HP_BUFS))
        work = ctx.enter_context(tc.tile_pool(name="work", bufs=WORK_BUFS))
        ohp = ctx.enter_context(tc.tile_pool(name="ohp", bufs=2))
        outp = ctx.enter_context(tc.tile_pool(name="outp", bufs=2))
        psum = ctx.enter_context(tc.tile_pool(name="psum", bufs=1, space="PSUM"))

        # ---- load constants ----
        lat_sb = const.tile([128, BC], bf16, tag="lat")
        nc.sync.dma_start(out=lat_sb[:], in_=lat[:])
        whh_sb = const.tile([128, 2, G3], fp8, tag="whh")
        nc.sync.dma_start(out=whh_sb[:], in_=whh[:])
        giv_sb = const.tile([16, 2, G3 + H], fp8, tag="giv")
        nc.sync.dma_start(out=giv_sb[:], in_=giv[:])
        wd0_sb = const.tile([128, H], bf16, tag="wd0")
        nc.sync.dma_start(out=wd0_sb[:], in_=wd0[:])
        wd1_sb = const.tile([128, 2, H], bf16, tag="wd1")
        nc.sync.dma_start(out=wd1_sb[:], in_=wd1[:])
        wd2_sb = const.tile([128, 2, H], bf16, tag="wd2")
        nc.sync.dma_start(out=wd2_sb[:], in_=wd2[:])
        wp1_sb = const.tile([128, 2, A], bf16, tag="wp1")
        nc.sync.dma_start(out=wp1_sb[:], in_=wp1[:])
        wp2_sb = const.tile([128, A], bf16, tag="wp2")
        nc.sync.dma_start(out=wp2_sb[:], in_=wp2[:])
        bias_sb = const.tile([128, 10], f32, tag="bias")
        nc.sync.dma_start(out=bias_sb[:], in_=bias[:])
        iden_sb = const.tile([128, 128], bf16, tag="iden")
        nc.sync.dma_start(out=iden_sb[:], in_=iden[:])

        # bias_sb columns: 0,1: b_hh_n (m0, m1); 2: bp1 (x4); 3: bp2 (x4);
        # 4..9: bd0 m0, bd0 m1, bd1 m0, bd1 m1, bd2 m0, bd2 m1

        # ---- MLP prologue (bf16): h0 = mlp(latent) ----
        h_cur = hp.tile([128, 2, BC], bf16, tag="h", name="h_init")
        h8_cur = hp.tile([128, 2, BC], fp8, tag="h8", name="h8_init")
        h1 = work.tile([128, 2, BC], bf16, tag="mlpa", name="mlpa")
        h2 = work.tile([128, 2, BC], bf16, tag="mlpb", name="mlpb")
        for c in range(NCH):
            cs = slice(c * CH, (c + 1) * CH)
            ps = psum.tile([128, 2, CH], f32, tag="pr", bufs=2, name=f"mlp1_{c}")
            for m in range(2):
                nc.tensor.matmul(
                    ps[:, m, :], wd0_sb[:, m * 128:(m + 1) * 128], lat_sb[:, cs],
                    start=True, stop=True)
            for m in range(2):
                nc.scalar.activation(
                    out=h1[:, m, cs], in_=ps[:, m, :], func=Act.Relu,
                    bias=bias_sb[:, 4 + m:5 + m])
        for c in range(NCH):
            cs = slice(c * CH, (c + 1) * CH)
            ps = psum.tile([128, 2, CH], f32, tag="pr", bufs=2, name=f"mlp2_{c}")
            for m in range(2):
                for kc in range(2):
                    nc.tensor.matmul(
                        ps[:, m, :], wd1_sb[:, kc, m * 128:(m + 1) * 128],
                        h1[:, kc, cs], start=(kc == 0), stop=(kc == 1))
            for m in range(2):
                nc.scalar.activation(
                    out=h2[:, m, cs], in_=ps[:, m, :], func=Act.Relu,
                    bias=bias_sb[:, 6 + m:7 + m])
        for c in range(NCH):
            cs = slice(c * CH, (c + 1) * CH)
            ps = psum.tile([128, 2, CH], f32, tag="pr", bufs=2, name=f"mlp3_{c}")
            for m in range(2):
                for kc in range(2):
                    nc.tensor.matmul(
                        ps[:, m, :], wd2_sb[:, kc, m * 128:(m + 1) * 128],
                        h2[:, kc, cs], start=(kc == 0), stop=(kc == 1))
            for m in range(2):
                nc.scalar.activation(
                    out=h_cur[:, m, cs], in_=ps[:, m, :], func=Act.Identity,
                    bias=bias_sb[:, 8 + m:9 + m])
            nc.vector.tensor_copy(h8_cur[:, :, cs], h_cur[:, :, cs])

        # ---- GRU steps ----
        def emit_proj_p1(h_tile, tp):
            """p1 matmuls for step tp (pipelined 1 behind): out [32, 8, 128]."""
            p1ps = psum.tile([32, 8, 128], f32, tag=PROJ_TAG, bufs=2,
                             name=f"p1ps_{tp}")
            for j in range(8):
                bs = slice(j * 128, (j + 1) * 128)
                for kc in range(2):
                    nc.tensor.matmul(
                        p1ps[:, j, :], wp1_sb[:, kc, :], h_tile[:, kc, bs],
                        start=(kc == 0), stop=(kc == 1))
            return p1ps

        def emit_proj_relu(p1ps, tp):
            p1t = work.tile([32, 8, 128], bf16, tag="p1t", name=f"p1t_{tp}")
            if RELU_ENG == "act":
                nc.scalar.activation(out=p1t[:], in_=p1ps[:], func=Act.Relu,
                                     bias=bias_sb[0:32, 2:3])
            elif RELU_ENG == "split":
                nc.scalar.activation(out=p1t[:, 0:4, :], in_=p1ps[:, 0:4, :],
                                     func=Act.Relu, bias=bias_sb[0:32, 2:3])
                nc.vector.tensor_scalar(
                    out=p1t[:, 4:8, :], in0=p1ps[:, 4:8, :],
                    scalar1=bias_sb[0:32, 2:3], scalar2=0.0,
                    op0=Alu.add, op1=Alu.max)
            else:
                nc.vector.tensor_scalar(
                    out=p1t[:], in0=p1ps[:], scalar1=bias_sb[0:32, 2:3],
                    scalar2=0.0, op0=Alu.add, op1=Alu.max)
            return p1t

        def emit_proj_p2(p1t, tp, stage):
            """p2 matmuls + DVE stage copy for step tp."""
            p2ps = psum.tile([128, 8, A], f32, tag=PROJ_TAG, bufs=2,
                             name=f"p2ps_{tp}")
            for j in range(8):
                nc.tensor.matmul(
                    p2ps[:, j, :], p1t[:, j, :], wp2_sb[0:A, :],
                    start=True, stop=True)
            if STAGE_ACT:
                nc.scalar.activation(
                    out=stage[:, tp % NST, :],
                    in_=p2ps.rearrange("p j a -> p (j a)"), func=Act.Identity)
            else:
                nc.vector.tensor_copy(
                    stage[:, tp % NST, :], p2ps.rearrange("p j a -> p (j a)"))

        def gate_mms(pr, pz, phn, pin, oh_rhs, h8_rhs):
            for m in range(2):
                nc.tensor.matmul(
                    pr[:, m, :], giv_sb[:, :, m * 128:(m + 1) * 128],
                    oh_rhs, start=True, stop=False, perf_mode=PM.DoubleRow)
                nc.tensor.matmul(
                    pr[:, m, :], whh_sb[:, :, m * 128:(m + 1) * 128],
                    h8_rhs, start=False, stop=True, perf_mode=PM.DoubleRow)
            for m in range(2):
                nc.tensor.matmul(
                    pz[:, m, :], giv_sb[:, :, 256 + m * 128:256 + (m + 1) * 128],
                    oh_rhs, start=True, stop=False, perf_mode=PM.DoubleRow)
                nc.tensor.matmul(
                    pz[:, m, :], whh_sb[:, :, 256 + m * 128:256 + (m + 1) * 128],
                    h8_rhs, start=False, stop=True, perf_mode=PM.DoubleRow)
            for m in range(2):
                nc.tensor.matmul(
                    phn[:, m, :], whh_sb[:, :, 512 + m * 128:512 + (m + 1) * 128],
                    h8_rhs, start=True, stop=False, perf_mode=PM.DoubleRow)
                nc.tensor.matmul(
                    phn[:, m, :], giv_sb[:, :, 768 + m * 128:768 + (m + 1) * 128],
                    oh_rhs, start=False, stop=True, perf_mode=PM.DoubleRow)
            for m in range(2):
                nc.tensor.matmul(
                    pin[:, m, :], giv_sb[:, :, 512 + m * 128:512 + (m + 1) * 128],
                    oh_rhs, start=True, stop=False, perf_mode=PM.DoubleRow)

        h_prev = None
        p1t_prev = None
        oh_g = None
        stages = {}
        for t in range(T):
            g, ti = divmod(t, NST)
            if ti == 0:
                oh_g = ohp.tile([16, 2, NST, BC], fp8, tag="ohg",
                                name=f"ohg_{g}")
                nc.sync.dma_start(out=oh_g[:], in_=oh[:, :, g * NST:(g + 1) * NST, :])
                stages[g] = outp.tile([128, NST, NCH * 4 * A], f32, tag="stage",
                                      name=f"stage_{g}")

            h_new = hp.tile([128, 2, BC], bf16, tag="h", name=f"h_{t}")
            h8_new = hp.tile([128, 2, BC], fp8, tag="h8", name=f"h8_{t}")
            rz = work.tile([128, 4, BC], bf16, tag="rz", name=f"rz_{t}")
            npre = work.tile([128, 2, BC], bf16, tag="npre", name=f"npre_{t}")
            nsb = work.tile([128, 2, BC], bf16, tag="nsb", name=f"nsb_{t}")
            t3 = work.tile([128, 2, BC], bf16, tag="t3", name=f"t3_{t}")
            t4 = work.tile([128, 2, BC], bf16, tag="t4", name=f"t4_{t}")
            c0s, c1s = slice(0, CH), slice(CH, BC)

            if P1_LEAD and h_prev is not None:
                p1ps_prev = emit_proj_p1(h_prev, t - 1)

            # NOTE: z weights are negated host-side -> sigmoid yields
            # zc = 1-z:  h_new = zc*n + (h - zc*h)   (c0, early q/s)
            #            h_new = h - zc*(h - n)      (c1, post-tanh)
            pr0 = psum.tile([128, 2, CH], f32, tag="pr", bufs=2, name=f"pr_{t}_0")
            pz0 = psum.tile([128, 2, CH], f32, tag="pr", bufs=2, name=f"pz_{t}_0")
            phn0 = psum.tile([128, 2, CH], f32, tag="pnp", bufs=2,
                             name=f"phn_{t}_0")
            pin0 = psum.tile([128, 2, CH], f32, tag="pnp", bufs=2,
                             name=f"pin_{t}_0")
            gate_mms(pr0, pz0, phn0, pin0, oh_g[:, :, ti, c0s],
                     h8_cur[:, :, c0s])
            pr1 = psum.tile([128, 2, CH], f32, tag="pr", bufs=2, name=f"pr_{t}_1")
            pz1 = psum.tile([128, 2, CH], f32, tag="pr", bufs=2, name=f"pz_{t}_1")
            phn1 = psum.tile([128, 2, CH], f32, tag="pnp", bufs=2,
                             name=f"phn_{t}_1")
            pin1 = psum.tile([128, 2, CH], f32, tag="pnp", bufs=2,
                             name=f"pin_{t}_1")
            gate_mms(pr1, pz1, phn1, pin1, oh_g[:, :, ti, c1s],
                     h8_cur[:, :, c1s])
            # proj(t-1) p1 matmuls go behind the gate matmuls on the PE
            if not P1_LEAD and h_prev is not None:
                p1ps_prev = emit_proj_p1(h_prev, t - 1)

            def npre_c(ci, phn, cs):
                nc.vector.scalar_tensor_tensor(
                    out=npre[:, :, cs], in0=phn[:], scalar=0.0,
                    in1=rz[:, 0:2, cs], op0=Alu.add, op1=Alu.mult)

            def ident_c(pin, cs):
                for m in range(2):
                    nc.tensor.matmul(pin[:, m, :], iden_sb[:], npre[:, m, cs],
                                     start=False, stop=True)

            def sig_r0():
                nc.scalar.activation(out=rz[:, 0:2, c0s], in_=pr0[:], func=Act.Sigmoid)
            def sig_z0():
                nc.scalar.activation(out=rz[:, 2:4, c0s], in_=pz0[:], func=Act.Sigmoid)
            def sig_r1():
                nc.scalar.activation(out=rz[:, 0:2, c1s], in_=pr1[:], func=Act.Sigmoid)
            def sig_z1():
                nc.scalar.activation(out=rz[:, 2:4, c1s], in_=pz1[:], func=Act.Sigmoid)
            def tanh0():
                npre_c(0, phn0, c0s)
                ident_c(pin0, c0s)
                nc.scalar.activation(out=nsb[:, :, c0s], in_=pin0[:], func=Act.Tanh)
            def tanh1():
                npre_c(1, phn1, c1s)
                ident_c(pin1, c1s)
                if M1SPLIT:
                    for m in range(2):
                        nc.scalar.activation(out=nsb[:, m, c1s],
                                             in_=pin1[:, m, :], func=Act.Tanh)
                else:
                    nc.scalar.activation(out=nsb[:, :, c1s], in_=pin1[:],
                                         func=Act.Tanh)

            ORDERS = {
                "A": [sig_r0, sig_z0, tanh0, sig_r1, sig_z1, tanh1],
                "B": [sig_r0, sig_z0, sig_r1, tanh0, sig_z1, tanh1],
                "C": [sig_r0, sig_z0, sig_r1, sig_z1, tanh0, tanh1],
                "D": [sig_r0, sig_r1, sig_z0, sig_z1, tanh0, tanh1],
                "E": [sig_r0, sig_r1, sig_z0, tanh0, sig_z1, tanh1],
            }
            for fn in ORDERS[ACT_ORDER]:
                fn()

            # chunk 0 combine
            def combine(form, cs_, conv_gps):
                if form == 5:
                    # q = zc*h, s = h - q (early);  u = zc*n, h = u + s (late)
                    if QS0_GPS:
                        nc.gpsimd.scalar_tensor_tensor(
                            out=t3[:, :, cs_], in0=h_cur[:, :, cs_], scalar=0.0,
                            in1=rz[:, 2:4, cs_], op0=Alu.add, op1=Alu.mult)
                        nc.gpsimd.scalar_tensor_tensor(
                            out=t3[:, :, cs_], in0=h_cur[:, :, cs_], scalar=0.0,
                            in1=t3[:, :, cs_], op0=Alu.add, op1=Alu.subtract)
                    else:
                        nc.vector.tensor_mul(t3[:, :, cs_], rz[:, 2:4, cs_], h_cur[:, :, cs_])
                        nc.vector.tensor_sub(t3[:, :, cs_], h_cur[:, :, cs_], t3[:, :, cs_])
                    nc.vector.tensor_mul(t4[:, :, cs_], rz[:, 2:4, cs_], nsb[:, :, cs_])
                    if HN8C0:
                        nc.vector.tensor_add(h8_new[:, :, cs_], t4[:, :, cs_],
                                             t3[:, :, cs_])
                        nc.gpsimd.scalar_tensor_tensor(
                            out=h_new[:, :, cs_], in0=t4[:, :, cs_], scalar=0.0,
                            in1=t3[:, :, cs_], op0=Alu.add, op1=Alu.add)
                        return
                    nc.vector.tensor_add(h_new[:, :, cs_], t4[:, :, cs_], t3[:, :, cs_])
                else:
                    # h_new = h - zc*(h - n)
                    nc.vector.tensor_sub(t3[:, :, cs_], h_cur[:, :, cs_], nsb[:, :, cs_])
                    nc.vector.tensor_mul(t3[:, :, cs_], rz[:, 2:4, cs_], t3[:, :, cs_])
                    if HN8C1:
                        nc.vector.tensor_sub(h8_new[:, :, cs_], h_cur[:, :, cs_],
                                             t3[:, :, cs_])
                        nc.gpsimd.scalar_tensor_tensor(
                            out=h_new[:, :, cs_], in0=h_cur[:, :, cs_], scalar=0.0,
                            in1=t3[:, :, cs_], op0=Alu.add, op1=Alu.subtract)
                        return
                    nc.vector.tensor_sub(h_new[:, :, cs_], h_cur[:, :, cs_], t3[:, :, cs_])
                if conv_gps:
                    nc.gpsimd.tensor_copy(h8_new[:, :, cs_], h_new[:, :, cs_])
                else:
                    nc.vector.tensor_copy(h8_new[:, :, cs_], h_new[:, :, cs_])

            combine(FORM0, c0s, CONV0_GPS)

            # proj(t-1) p2 + stage
            if h_prev is not None:
                p1t_prev = emit_proj_relu(p1ps_prev, t - 1)
                emit_proj_p2(p1t_prev, t - 1, stages[(t - 1) // NST])
                if not DIRECT_DMA and (t - 1) % NST == NST - 1:
                    gg = (t - 1) // NST
                    nc.sync.dma_start(
                        out=outw[:, gg * NST:(gg + 1) * NST, :],
                        in_=stages[gg][:])

            # chunk 1 combine
            if M1SPLIT:
                for m in range(2):
                    ms = (slice(None), m, c1s)
                    nc.vector.tensor_sub(t3[ms], h_cur[ms], nsb[ms])
                    nc.vector.tensor_mul(t3[ms], rz[:, 2 + m, c1s], t3[ms])
                    nc.vector.tensor_sub(h_new[ms], h_cur[ms], t3[ms])
                    if CONV1_GPS:
                        nc.gpsimd.tensor_copy(h8_new[ms], h_new[ms])
                    else:
                        nc.vector.tensor_copy(h8_new[ms], h_new[ms])
            else:
                combine(FORM1, c1s, CONV1_GPS)

            h_prev = h_new
            h_cur = h_new
            h8_cur = h8_new
        p1ps_last = emit_proj_p1(h_prev, T - 1)
        p1t_last = emit_proj_relu(p1ps_last, T - 1)
        emit_proj_p2(p1t_last, T - 1, stages[NGRP - 1])
        if not DIRECT_DMA:
            nc.sync.dma_start(
                out=outw[:, (NGRP - 1) * NST:, :], in_=stages[NGRP - 1][:])

    nc.finalize()
    return nc


def _prep_inputs(latent, target, embed, W_ih, b_ih, W_hh, b_hh,
                 Wd0, bd0, Wd1, bd1, Wd2, bd2, Wp1, bp1, Wp2, bp2):
    f = np.float32
    latent = np.asarray(latent, dtype=f)
    embed = np.asarray(embed, dtype=f)
    W_ih = np.asarray(W_ih, dtype=f)
    b_ih = np.asarray(b_ih, dtype=f)
    W_hh = np.asarray(W_hh, dtype=f)
    b_hh = np.asarray(b_hh, dtype=f)

    # tokens with teacher-forcing shift
    tokens = np.concatenate(
        [np.zeros((B, 1), dtype=np.int64),
         np.asarray(target[:, :-1], dtype=np.int64)], axis=1)  # [B, T]

    # per-token gate table with biases folded in:
    #   r/z rows: giv + b_ih + b_hh ; n rows: giv + b_ih
    # plus H extra columns: b_hh_n broadcast to every token (accumulated
    # onto phn by the one-hot matmul). The z columns are NEGATED so the
    # sigmoid yields zc = 1 - z.
    giv = embed @ W_ih.T  # [A, 3H]
    gt = np.concatenate([giv, np.zeros((A, H), dtype=f)], axis=1)
    gt[:, :2 * H] += (b_ih + b_hh)[None, :2 * H]
    gt[:, 2 * H:3 * H] += b_ih[None, 2 * H:]
    gt[:, H:2 * H] *= -1.0
    gt[:, 3 * H:] = b_hh[None, 2 * H:]
    # DR layout [16, 2, 3H+H]: table row a = (a % 16) + 16 * (a // 16)
    giv_dr = np.ascontiguousarray(
        gt.reshape(2, 16, G3 + H).transpose(1, 0, 2)).astype(FP8)

    # one-hot, DR layout [16, 2, T, B]
    tok_tm = tokens.T  # [T, B]
    ohf = np.zeros((16, 2, T, B), dtype=FP8)
    for a in range(A):
        ohf[a % 16, a // 16][tok_tm == a] = 1.0

    whhT = np.ascontiguousarray(W_hh.T).copy()  # [H, 3H]
    whhT[:, H:2 * H] *= -1.0  # negated z gate -> sigmoid gives 1-z
    whh_dr = np.ascontiguousarray(
        whhT.reshape(2, 128, G3).transpose(1, 0, 2)).astype(FP8)

    wd0_l = np.asarray(Wd0, dtype=f).astype(BF16)                   # [128, 256]
    wd1_l = np.ascontiguousarray(
        np.asarray(Wd1, dtype=f).reshape(2, 128, H).transpose(1, 0, 2)).astype(BF16)
    wd2_l = np.ascontiguousarray(
        np.asarray(Wd2, dtype=f).reshape(2, 128, H).transpose(1, 0, 2)).astype(BF16)
    wp1_l = np.ascontiguousarray(
        np.asarray(Wp1, dtype=f).reshape(2, 128, A).transpose(1, 0, 2)).astype(BF16)
    wp2_l = np.zeros((128, A), dtype=f)
    wp2_l[:A] = np.asarray(Wp2, dtype=f)
    wp2_l = wp2_l.astype(BF16)                                      # [128, 32]

    bias_pack = np.zeros((128, 10), dtype=f)
    bias_pack[:, 0] = b_hh[2 * H: 2 * H + 128]
    bias_pack[:, 1] = b_hh[2 * H + 128:]
    bias_pack[:A, 2] = np.asarray(bp1, dtype=f)
    bias_pack[:, 3] = np.tile(np.asarray(bp2, dtype=f), 4)  # per (j,a) flattened
    bias_pack[:, 4] = np.asarray(bd0, dtype=f)[:128]
    bias_pack[:, 5] = np.asarray(bd0, dtype=f)[128:]
    bias_pack[:, 6] = np.asarray(bd1, dtype=f)[:128]
    bias_pack[:, 7] = np.asarray(bd1, dtype=f)[128:]
    bias_pack[:, 8] = np.asarray(bd2, dtype=f)[:128]
    bias_pack[:, 9] = np.asarray(bd2, dtype=f)[128:]

    iden = np.eye(128, dtype=f).astype(BF16)

    latT = np.ascontiguousarray(latent.T).astype(BF16)  # [128, B]

    shared = dict(whh=whh_dr, giv=giv_dr, wd0=wd0_l, wd1=wd1_l, wd2=wd2_l,
                  wp1=wp1_l, wp2=wp2_l, bias=bias_pack, iden=iden)
    in_maps = []
    for cid in range(NCORES):
        bs = slice(cid * BC, (cid + 1) * BC)
        m = dict(shared)
        m["lat"] = np.ascontiguousarray(latT[:, bs])
        m["oh"] = np.ascontiguousarray(ohf[:, :, :, bs])
        in_maps.append(m)
    return in_maps


def _unpack_out(outw, bp2):
    """outw [128, T, 8*A] f32 -> [BC, T, A] + bp2."""
    o = outw.reshape(128, T, 8, A)               # p, t, j, a
    o = o.transpose(2, 0, 1, 3)                  # j, p, t, a
    o = np.ascontiguousarray(o.reshape(BC, T, A))
    o += np.asarray(bp2, dtype=np.float32)[None, None, :]
    return o


def kernel(**inputs):
    from concourse.bass_utils import run_bass_kernel_spmd

    if "nc" not in _CACHE:
        _CACHE["nc"] = _build()
    nc = _CACHE["nc"]

    in_maps = _prep_inputs(**inputs)
    res = run_bass_kernel_spmd(nc, in_maps, core_ids=list(range(NCORES)))
    bp2 = inputs["bp2"]
    outs = [_unpack_out(r["outw"], bp2) for r in res.results]
    return np.concatenate(outs, axis=0).astype(np.float32)
